# revision 24
# baseline (speedup 1.0000x reference)
"""Trainium2 Bass kernel for nn_CausalConv1d (depthwise causal conv, K=4).

Reference computation (T=8192, C=8448, K=4):
    padded = concat([state, inputs], axis=0)            # [T+K, C]
    out[t, c] = bias[c] + sum_j padded[t+j, c] * weight[c, j]
    updated_state = inputs[T-K:T]
    returns (out[:, :8192], out[:, 8192:8320], out[:, 8320:8448], updated_state)

Strategy (measured ~125-130 us/core marginal exec on TRN2, DMA 78 us,
PE-bound; see test.py for methodology):
  - Shard T across the 8 cores: core m computes out rows [m*1024, (m+1)*1024),
    consuming padded rows [m*1024, m*1024+1024+K-1) (halo of K-1=3 rows).
  - fp16 on the wire: inputs are cast to fp16 on the host (10-bit mantissa,
    range-safe for this data) and outputs come back fp16 — halves HBM traffic
    for this memory-regime problem. l2 rel err vs the f32 reference ~3.4e-4.
  - Host pre-permutes each core's input to the exact SBUF layout
    x_flat[p, g*TINP + t] = padded[m*1024 + t, g*128 + p], so every DMA is a
    plain 2D transfer with long contiguous per-partition runs (the naive
    transposed-AP DMA was ~2.4x slower).
  - On device, the depthwise conv runs on the TensorEngine: for each group of
    128 channels, 4 matmuls with lhsT = diag(weight[:, j]) and rhs = the input
    tile read at free-offset j, accumulating the 4 taps into a 2-bank PSUM
    tile. j-outer order so consecutive matmuls share stationary weights
    (fewer LDWEIGHTS). ScalarE then does one 1024-wide PSUM->SBUF copy per
    group with the bias fused (activation Identity, per-partition bias AP).
  - Diagonal weight matrices are built on the fly by the VectorE from a host
    identity matrix via tensor_scalar_mul (per-partition scalar = w[:, j]).
  - 13 of the 66 channel groups (gg % 5 == 4) bypass the PE and are computed
    elementwise (ACT does tap0+bias, DVE does taps 1-3 and the adds) to
    offload the PE, which is otherwise the critical path.
  - Input DMAs issue on the SP HWDGE ring (nc.sync), output DMAs on the ACT
    ring (nc.scalar) so the two streams don't serialize on one FIFO.
  - updated_state is a pure host-side slice of the inputs.
  - A fp32r variant (dtype='f32r': fp32 rounded to 11 mantissa bits, 1 PE
    cycle/row) is kept as a higher-precision fallback (~1.3e-4 l2 rel err,
    ~237 us).
"""

import sys

sys.path.insert(0, "/opt/trn_rl_repo")

from contextlib import ExitStack

import numpy as np

import concourse.bass as bass
import concourse.mybir as mybir
from concourse import bacc
from concourse.tile import TileContext
from concourse.bass_utils import run_bass_kernel_spmd

# Problem shapes (hardcoded per the harness contract).
T, C, K = 8192, 8448, 4
N_CORES = 8
T_LOC = T // N_CORES  # 1024 output rows per core
TIN = T_LOC + K - 1  # 1027 input rows per core
TINP = TIN + 1  # padded per-group stride so tap offsets stay 4B-aligned (fp16)
P = 128  # SBUF partitions
G = C // P  # 66 channel groups
GB = 6  # channel groups per DMA batch
NI = G // GB  # 11 iterations
CHUNK = 512  # PSUM bank = 512 fp32
OUTPUT_DIMS = (8192, 128, 128)

_F32 = mybir.dt.float32
_F32R = mybir.dt.float32r


def _round_fp32r(a: np.ndarray) -> np.ndarray:
    """Round fp32 to 11 mantissa bits (round-to-nearest-even) — exactly what
    the PE's fp32r datapath does to its inputs (verified on hardware)."""
    ai = np.ascontiguousarray(a, dtype=np.float32).view(np.uint32)
    shift = 23 - 11
    bias = ((ai >> np.uint32(shift)) & np.uint32(1)) + np.uint32((1 << (shift - 1)) - 1)
    return (((ai + bias) >> np.uint32(shift)) << np.uint32(shift)).view(np.float32)


def _build(loop_reps: int | None = None, mode: str = "full", dtype: str = "fp16", vec_groups: bool = True, drains: str = "act", gb: int = GB, hints: bool = False, bufs=(3, 2, 16)):
    """Build the per-core Bass program.

    loop_reps (timing only): wrap the body in a device-side For_i repeat so
    per-exec dispatch overhead can be subtracted out when benchmarking.
    mode: 'full' | 'dma' (skip compute, out-DMA echoes input tile) |
          'compute' (load one tile, run all compute on it, one out-DMA).
    dtype: 'fp16' (x/y/diag in fp16 — half the HBM traffic, 10-bit mantissa)
           or 'f32r' (x/diag fp32r, y fp32 — 11-bit mantissa products).
    """
    nc = bacc.Bacc("TRN2", target_bir_lowering=False, debug=False)

    if dtype == "fp16":
        xdt = ydt = mybir.dt.float16
    else:
        xdt, ydt = _F32R, _F32

    x = nc.dram_tensor("x", [P, G * TINP], xdt, kind="ExternalInput")
    w = nc.dram_tensor("w", [P, G * K], _F32, kind="ExternalInput")
    b = nc.dram_tensor("b", [P, G], _F32, kind="ExternalInput")
    ident = nc.dram_tensor("ident", [P, P], _F32, kind="ExternalInput")
    y = nc.dram_tensor("y", [P, G * T_LOC], ydt, kind="ExternalOutput")

    # Some channel groups are computed elementwise on ACT+DVE instead of the
    # PE, to balance the compute engines (PE alone would be the ceiling).
    # vec groups: gg % 5 == 4 (13 of 66). PE-group drains (PSUM -> SBUF +
    # bias) all go to ACT by default ('split' sends gg % 5 == 0 to DVE).
    def is_vec(gg):
        return vec_groups and dtype == "fp16" and mode == "full" and gg % 5 == 4

    def dve_drains(gg):
        return drains == "split" and gg % 5 == 0

    act_t = mybir.ActivationFunctionType

    with TileContext(nc) as tc:
        with (
            tc.tile_pool(name="const", bufs=1) as cpool,
            tc.tile_pool(name="xin", bufs=bufs[0]) as xpool,
            tc.tile_pool(name="yout", bufs=bufs[1]) as ypool,
            tc.tile_pool(name="diag", bufs=bufs[2]) as dpool,
            tc.tile_pool(name="vec", bufs=4) as vpool,
            tc.tile_pool(name="psum", bufs=4, space=bass.MemorySpace.PSUM) as pspool,
        ):
            wt = cpool.tile([P, G * K], _F32)
            bt = cpool.tile([P, G], _F32)
            it = cpool.tile([P, P], _F32)
            nc.sync.dma_start(out=wt[:], in_=w[:])
            nc.sync.dma_start(out=bt[:], in_=b[:])
            nc.sync.dma_start(out=it[:], in_=ident[:])

            with ExitStack() as loop_ctx:
                if loop_reps is not None:
                    hint = (
                        (mybir.EngineType.PE, mybir.EngineType.DVE)
                        if hints
                        else ()
                    )
                    loop_ctx.enter_context(
                        tc.For_i(0, loop_reps, 1, hint_engines=hint)
                    )
                for i in range(G // gb):
                    if mode == "compute" and i > 0:
                        xt = xt  # noqa: PLW0127 — reuse first tile
                    else:
                        xt = xpool.tile([P, gb * TINP], xdt)
                        nc.sync.dma_start(
                            out=xt[:], in_=x[:, i * gb * TINP : (i + 1) * gb * TINP]
                        )
                    if mode == "dma":
                        nc.scalar.dma_start(
                            out=y[:, i * gb * T_LOC : (i + 1) * gb * T_LOC],
                            in_=xt[:, : gb * T_LOC].bitcast(ydt),
                        )
                        continue
                    yt = ypool.tile([P, gb * T_LOC], ydt)
                    for g in range(gb):
                        gg = i * gb + g
                        base = g * TINP
                        yslc = yt[:, g * T_LOC : (g + 1) * T_LOC]
                        bias_ap = bt[:, gg : gg + 1]
                        if is_vec(gg):
                            # Vector path: 4 taps + bias via ACT+DVE, full
                            # 1024-wide ops to amortize fixed overheads.
                            w_ap = [
                                wt[:, gg * K + j : gg * K + j + 1] for j in range(K)
                            ]
                            xs = [
                                xt[:, base + j : base + j + T_LOC] for j in range(K)
                            ]
                            a0 = vpool.tile([P, T_LOC], ydt, name="a0", tag="a0")
                            nc.scalar.activation(
                                a0[:], xs[0], act_t.Identity,
                                bias=bias_ap, scale=w_ap[0],
                            )
                            a1 = vpool.tile([P, T_LOC], ydt, name="a1", tag="a1")
                            nc.vector.tensor_scalar_mul(a1[:], xs[1], w_ap[1])
                            m2 = vpool.tile([P, T_LOC], ydt, name="m2", tag="m2")
                            nc.vector.tensor_scalar_mul(m2[:], xs[2], w_ap[2])
                            m3 = vpool.tile([P, T_LOC], ydt, name="m3", tag="m3")
                            nc.vector.tensor_scalar_mul(m3[:], xs[3], w_ap[3])
                            s1 = vpool.tile([P, T_LOC], ydt, name="s1", tag="s1")
                            nc.vector.tensor_add(s1[:], a0[:], a1[:])
                            s2 = vpool.tile([P, T_LOC], ydt, name="s2", tag="s2")
                            nc.vector.tensor_add(s2[:], m2[:], m3[:])
                            nc.vector.tensor_add(yslc, s1[:], s2[:])
                            continue
                        diags = []
                        for j in range(K):
                            d = dpool.tile([P, P], xdt, tag="diag")
                            nc.vector.tensor_scalar_mul(
                                d[:], it[:], wt[:, gg * K + j : gg * K + j + 1]
                            )
                            diags.append(d)
                        # One 2-bank PSUM tile per group; j-outer so consecutive
                        # matmuls share the same stationary weights (less LDW).
                        ps = pspool.tile([P, T_LOC], _F32, name="ps", tag="ps")
                        for j in range(K):
                            for c in range(T_LOC // CHUNK):
                                nc.tensor.matmul(
                                    ps[:, c * CHUNK : (c + 1) * CHUNK],
                                    diags[j][:],
                                    xt[:, base + c * CHUNK + j : base + c * CHUNK + j + CHUNK],
                                    start=(j == 0),
                                    stop=(j == K - 1),
                                    skip_group_check=True,
                                )
                        # Single 1024-wide drain (PSUM+bias -> SBUF), split
                        # between DVE and ACT to balance the engines.
                        if dve_drains(gg):
                            nc.vector.tensor_scalar_add(yslc, ps[:], bias_ap)
                        else:
                            nc.scalar.add(yslc, ps[:], bias_ap)
                    if mode == "compute" and i < G // gb - 1:
                        continue
                    nc.scalar.dma_start(
                        out=y[:, i * gb * T_LOC : (i + 1) * gb * T_LOC], in_=yt[:]
                    )
    nc.compile()
    return nc


DTYPE = "fp16"  # 'fp16' or 'f32r'

_NC_CACHE = None


def _get_nc():
    global _NC_CACHE
    if _NC_CACHE is None:
        _NC_CACHE = _build(dtype=DTYPE)
    return _NC_CACHE


def _prepare_in_maps(inputs, state, weight, bias):
    inputs = np.asarray(inputs, dtype=np.float32)
    state = np.asarray(state, dtype=np.float32)
    weight = np.asarray(weight, dtype=np.float32)
    bias = np.asarray(bias, dtype=np.float32)

    padded = np.concatenate([state, inputs], axis=0)  # [T+K, C]
    if DTYPE == "fp16":
        padded_r = padded.astype(np.float16)
        w_r = weight  # diag build rounds to fp16 on the DVE write
    else:
        padded_r = _round_fp32r(padded)
        w_r = _round_fp32r(weight)

    w_sb = np.ascontiguousarray(
        w_r.reshape(G, P, K).transpose(1, 0, 2).reshape(P, G * K)
    )
    b_sb = np.ascontiguousarray(bias.reshape(G, P).T)  # [P, G]
    ident = np.eye(P, dtype=np.float32)

    in_maps = []
    for m in range(N_CORES):
        seg = padded_r[m * T_LOC : m * T_LOC + TIN]  # [TIN, C]
        # x_flat[p, g*TINP + t] = seg[t, g*128 + p]  (last TINP-TIN cols zero pad)
        xm = np.zeros((P, G, TINP), dtype=padded_r.dtype)
        xm[:, :, :TIN] = seg.reshape(TIN, G, P).transpose(2, 1, 0)
        in_maps.append(
            {"x": xm.reshape(P, G * TINP), "w": w_sb, "b": b_sb, "ident": ident}
        )
    return in_maps


def _assemble_out(results):
    """results[m]['y'] is [P, G*T_LOC] with y[p, g*T_LOC + t] = out[m*T_LOC+t, g*128+p]."""
    out = np.empty((T, C), dtype=np.float32)
    for m in range(N_CORES):
        yf = results[m]["y"].astype(np.float32).reshape(P, G, T_LOC)
        out[m * T_LOC : (m + 1) * T_LOC] = yf.transpose(2, 1, 0).reshape(T_LOC, C)
    return out


def kernel(inputs, state, weight, bias):
    nc = _get_nc()
    in_maps = _prepare_in_maps(inputs, state, weight, bias)
    res = run_bass_kernel_spmd(nc, in_maps, core_ids=list(range(N_CORES)))
    out = _assemble_out(res.results)

    split_points = np.cumsum(OUTPUT_DIMS[:-1]).tolist()
    split_outputs = tuple(
        np.ascontiguousarray(s) for s in np.split(out, split_points, axis=-1)
    )
    updated_state = np.ascontiguousarray(np.asarray(inputs, dtype=np.float32)[T - K : T])
    return (*split_outputs, updated_state)


# revision 29
# speedup vs baseline: 1.0103x; 1.0103x over previous
"""Trainium2 Bass kernel for nn_CausalConv1d (depthwise causal conv, K=4).

Reference computation (T=8192, C=8448, K=4):
    padded = concat([state, inputs], axis=0)            # [T+K, C]
    out[t, c] = bias[c] + sum_j padded[t+j, c] * weight[c, j]
    updated_state = inputs[T-K:T]
    returns (out[:, :8192], out[:, 8192:8320], out[:, 8320:8448], updated_state)

Strategy (measured ~125-130 us/core marginal exec on TRN2, DMA 78 us,
PE-bound; see test.py for methodology):
  - Shard T across the 8 cores: core m computes out rows [m*1024, (m+1)*1024),
    consuming padded rows [m*1024, m*1024+1024+K-1) (halo of K-1=3 rows).
  - fp16 on the wire: inputs are cast to fp16 on the host (10-bit mantissa,
    range-safe for this data) and outputs come back fp16 — halves HBM traffic
    for this memory-regime problem. l2 rel err vs the f32 reference ~3.4e-4.
  - Host pre-permutes each core's input to the exact SBUF layout
    x_flat[p, g*TINP + t] = padded[m*1024 + t, g*128 + p], so every DMA is a
    plain 2D transfer with long contiguous per-partition runs (the naive
    transposed-AP DMA was ~2.4x slower).
  - On device, the depthwise conv runs on the TensorEngine: for each group of
    128 channels, 4 matmuls with lhsT = diag(weight[:, j]) and rhs = the input
    tile read at free-offset j, accumulating the 4 taps into a 2-bank PSUM
    tile. j-outer order so consecutive matmuls share stationary weights
    (fewer LDWEIGHTS). ScalarE then does one 1024-wide PSUM->SBUF copy per
    group with the bias fused (activation Identity, per-partition bias AP).
  - Diagonal weight matrices are built on the fly by the VectorE from a host
    identity matrix via tensor_scalar_mul (per-partition scalar = w[:, j]).
  - 13 of the 66 channel groups (gg % 5 == 4) bypass the PE and are computed
    elementwise (ACT does tap0+bias, DVE does taps 1-3 and the adds) to
    offload the PE, which is otherwise the critical path.
  - Input DMAs issue on the SP HWDGE ring (nc.sync), output DMAs on the ACT
    ring (nc.scalar) so the two streams don't serialize on one FIFO.
  - updated_state is a pure host-side slice of the inputs.
  - A fp32r variant (dtype='f32r': fp32 rounded to 11 mantissa bits, 1 PE
    cycle/row) is kept as a higher-precision fallback (~1.3e-4 l2 rel err,
    ~237 us).
"""

import sys

sys.path.insert(0, "/opt/trn_rl_repo")

from contextlib import ExitStack

import numpy as np

import concourse.bass as bass
import concourse.mybir as mybir
from concourse import bacc
from concourse.tile import TileContext
from concourse.bass_utils import run_bass_kernel_spmd

# Problem shapes (hardcoded per the harness contract).
T, C, K = 8192, 8448, 4
N_CORES = 8
T_LOC = T // N_CORES  # 1024 output rows per core
TIN = T_LOC + K - 1  # 1027 input rows per core
TINP = TIN + 1  # padded per-group stride so tap offsets stay 4B-aligned (fp16)
P = 128  # SBUF partitions
G = C // P  # 66 channel groups
GB = 6  # channel groups per DMA batch
NI = G // GB  # 11 iterations
CHUNK = 512  # PSUM bank = 512 fp32
OUTPUT_DIMS = (8192, 128, 128)

_F32 = mybir.dt.float32
_F32R = mybir.dt.float32r


def _round_fp32r(a: np.ndarray) -> np.ndarray:
    """Round fp32 to 11 mantissa bits (round-to-nearest-even) — exactly what
    the PE's fp32r datapath does to its inputs (verified on hardware)."""
    ai = np.ascontiguousarray(a, dtype=np.float32).view(np.uint32)
    shift = 23 - 11
    bias = ((ai >> np.uint32(shift)) & np.uint32(1)) + np.uint32((1 << (shift - 1)) - 1)
    return (((ai + bias) >> np.uint32(shift)) << np.uint32(shift)).view(np.float32)


def _build(loop_reps: int | None = None, mode: str = "full", dtype: str = "fp16", vec_groups: bool = True, drains: str = "act", gb: int = GB, hints: bool = False, bufs=(3, 2, 16), vec_sel: int = 5, out_eng: str = "act", ps_chunks: int = 2, vec_first: bool = True, vec_tree: str = "chain"):
    """Build the per-core Bass program.

    loop_reps (timing only): wrap the body in a device-side For_i repeat so
    per-exec dispatch overhead can be subtracted out when benchmarking.
    mode: 'full' | 'dma' (skip compute, out-DMA echoes input tile) |
          'compute' (load one tile, run all compute on it, one out-DMA).
    dtype: 'fp16' (x/y/diag in fp16 — half the HBM traffic, 10-bit mantissa)
           or 'f32r' (x/diag fp32r, y fp32 — 11-bit mantissa products).
    """
    nc = bacc.Bacc("TRN2", target_bir_lowering=False, debug=False)

    if dtype == "fp16":
        xdt = ydt = mybir.dt.float16
    else:
        xdt, ydt = _F32R, _F32

    x = nc.dram_tensor("x", [P, G * TINP], xdt, kind="ExternalInput")
    w = nc.dram_tensor("w", [P, G * K], _F32, kind="ExternalInput")
    b = nc.dram_tensor("b", [P, G], _F32, kind="ExternalInput")
    ident = nc.dram_tensor("ident", [P, P], _F32, kind="ExternalInput")
    y = nc.dram_tensor("y", [P, G * T_LOC], ydt, kind="ExternalOutput")

    # Some channel groups are computed elementwise on ACT+DVE instead of the
    # PE, to balance the compute engines (PE alone would be the ceiling).
    # vec groups: gg % 5 == 4 (13 of 66). PE-group drains (PSUM -> SBUF +
    # bias) all go to ACT by default ('split' sends gg % 5 == 0 to DVE).
    def is_vec(gg):
        return vec_groups and dtype == "fp16" and mode == "full" and gg % vec_sel == vec_sel - 1

    def dve_drains(gg):
        return drains == "split" and gg % 5 == 0

    act_t = mybir.ActivationFunctionType

    with TileContext(nc) as tc:
        with (
            tc.tile_pool(name="const", bufs=1) as cpool,
            tc.tile_pool(name="xin", bufs=bufs[0]) as xpool,
            tc.tile_pool(name="yout", bufs=bufs[1]) as ypool,
            tc.tile_pool(name="diag", bufs=bufs[2]) as dpool,
            tc.tile_pool(name="vec", bufs=4) as vpool,
            tc.tile_pool(name="psum", bufs=8 // ps_chunks, space=bass.MemorySpace.PSUM) as pspool,
        ):
            wt = cpool.tile([P, G * K], _F32)
            bt = cpool.tile([P, G], _F32)
            it = cpool.tile([P, P], _F32)
            nc.sync.dma_start(out=wt[:], in_=w[:])
            nc.sync.dma_start(out=bt[:], in_=b[:])
            nc.sync.dma_start(out=it[:], in_=ident[:])

            with ExitStack() as loop_ctx:
                if loop_reps is not None:
                    hint = (
                        (mybir.EngineType.PE, mybir.EngineType.DVE)
                        if hints
                        else ()
                    )
                    loop_ctx.enter_context(
                        tc.For_i(0, loop_reps, 1, hint_engines=hint)
                    )
                for i in range(G // gb):
                    if mode == "compute" and i > 0:
                        xt = xt  # noqa: PLW0127 — reuse first tile
                    else:
                        xt = xpool.tile([P, gb * TINP], xdt)
                        nc.sync.dma_start(
                            out=xt[:], in_=x[:, i * gb * TINP : (i + 1) * gb * TINP]
                        )
                    if mode == "dma":
                        nc.scalar.dma_start(
                            out=y[:, i * gb * T_LOC : (i + 1) * gb * T_LOC],
                            in_=xt[:, : gb * T_LOC].bitcast(ydt),
                        )
                        continue
                    yt = ypool.tile([P, gb * T_LOC], ydt)
                    g_order = sorted(
                        range(gb), key=lambda gx: 0 if is_vec(i * gb + gx) else 1
                    ) if vec_first else range(gb)
                    for g in g_order:
                        gg = i * gb + g
                        base = g * TINP
                        yslc = yt[:, g * T_LOC : (g + 1) * T_LOC]
                        bias_ap = bt[:, gg : gg + 1]
                        if is_vec(gg):
                            # Vector path: 4 taps + bias via ACT+DVE, full
                            # 1024-wide ops to amortize fixed overheads.
                            w_ap = [
                                wt[:, gg * K + j : gg * K + j + 1] for j in range(K)
                            ]
                            xs = [
                                xt[:, base + j : base + j + T_LOC] for j in range(K)
                            ]
                            a0 = vpool.tile([P, T_LOC], ydt, name="a0", tag="a0")
                            nc.scalar.activation(
                                a0[:], xs[0], act_t.Identity,
                                bias=bias_ap, scale=w_ap[0],
                            )
                            a1 = vpool.tile([P, T_LOC], ydt, name="a1", tag="a1")
                            nc.vector.tensor_scalar_mul(a1[:], xs[1], w_ap[1])
                            m2 = vpool.tile([P, T_LOC], ydt, name="m2", tag="m2")
                            nc.vector.tensor_scalar_mul(m2[:], xs[2], w_ap[2])
                            m3 = vpool.tile([P, T_LOC], ydt, name="m3", tag="m3")
                            nc.vector.tensor_scalar_mul(m3[:], xs[3], w_ap[3])
                            if vec_tree == "chain":
                                # pure-DVE chain; ACT's a0 joins last so the
                                # cross-engine dependency is off the chain.
                                u = vpool.tile([P, T_LOC], ydt, name="u", tag="s1")
                                nc.vector.tensor_add(u[:], a1[:], m2[:])
                                v = vpool.tile([P, T_LOC], ydt, name="v", tag="s2")
                                nc.vector.tensor_add(v[:], u[:], m3[:])
                                nc.vector.tensor_add(yslc, v[:], a0[:])
                            else:
                                s1 = vpool.tile([P, T_LOC], ydt, name="s1", tag="s1")
                                nc.vector.tensor_add(s1[:], a0[:], a1[:])
                                s2 = vpool.tile([P, T_LOC], ydt, name="s2", tag="s2")
                                nc.vector.tensor_add(s2[:], m2[:], m3[:])
                                nc.vector.tensor_add(yslc, s1[:], s2[:])
                            continue
                        diags = []
                        for j in range(K):
                            d = dpool.tile([P, P], xdt, tag="diag")
                            nc.vector.tensor_scalar_mul(
                                d[:], it[:], wt[:, gg * K + j : gg * K + j + 1]
                            )
                            diags.append(d)
                        # PSUM tiles of ps_chunks banks per group; j-outer so
                        # consecutive matmuls share stationary weights (less LDW).
                        pw = ps_chunks * CHUNK
                        pss = [
                            pspool.tile([P, pw], _F32, name="ps", tag="ps")
                            for _ in range(T_LOC // pw)
                        ]
                        for j in range(K):
                            for c in range(T_LOC // CHUNK):
                                ps = pss[c // ps_chunks]
                                pc = c % ps_chunks
                                nc.tensor.matmul(
                                    ps[:, pc * CHUNK : (pc + 1) * CHUNK],
                                    diags[j][:],
                                    xt[:, base + c * CHUNK + j : base + c * CHUNK + j + CHUNK],
                                    start=(j == 0),
                                    stop=(j == K - 1),
                                    skip_group_check=True,
                                )
                        # Wide drains (PSUM+bias -> SBUF), DVE or ACT.
                        for pi, ps in enumerate(pss):
                            ysub = yt[:, g * T_LOC + pi * pw : g * T_LOC + (pi + 1) * pw]
                            if dve_drains(gg):
                                nc.vector.tensor_scalar_add(ysub, ps[:], bias_ap)
                            else:
                                nc.scalar.add(ysub, ps[:], bias_ap)
                    if mode == "compute" and i < G // gb - 1:
                        continue
                    out_dma = nc.gpsimd if out_eng == "gpsimd" else nc.scalar
                    out_dma.dma_start(
                        out=y[:, i * gb * T_LOC : (i + 1) * gb * T_LOC], in_=yt[:]
                    )
    nc.compile()
    return nc


DTYPE = "fp16"  # 'fp16' or 'f32r'

_NC_CACHE = None


def _get_nc():
    global _NC_CACHE
    if _NC_CACHE is None:
        _NC_CACHE = _build(dtype=DTYPE)
    return _NC_CACHE


def _prepare_in_maps(inputs, state, weight, bias):
    inputs = np.asarray(inputs, dtype=np.float32)
    state = np.asarray(state, dtype=np.float32)
    weight = np.asarray(weight, dtype=np.float32)
    bias = np.asarray(bias, dtype=np.float32)

    padded = np.concatenate([state, inputs], axis=0)  # [T+K, C]
    if DTYPE == "fp16":
        padded_r = padded.astype(np.float16)
        w_r = weight  # diag build rounds to fp16 on the DVE write
    else:
        padded_r = _round_fp32r(padded)
        w_r = _round_fp32r(weight)

    w_sb = np.ascontiguousarray(
        w_r.reshape(G, P, K).transpose(1, 0, 2).reshape(P, G * K)
    )
    b_sb = np.ascontiguousarray(bias.reshape(G, P).T)  # [P, G]
    ident = np.eye(P, dtype=np.float32)

    in_maps = []
    for m in range(N_CORES):
        seg = padded_r[m * T_LOC : m * T_LOC + TIN]  # [TIN, C]
        # x_flat[p, g*TINP + t] = seg[t, g*128 + p]  (last TINP-TIN cols zero pad)
        xm = np.zeros((P, G, TINP), dtype=padded_r.dtype)
        xm[:, :, :TIN] = seg.reshape(TIN, G, P).transpose(2, 1, 0)
        in_maps.append(
            {"x": xm.reshape(P, G * TINP), "w": w_sb, "b": b_sb, "ident": ident}
        )
    return in_maps


def _assemble_out(results):
    """results[m]['y'] is [P, G*T_LOC] with y[p, g*T_LOC + t] = out[m*T_LOC+t, g*128+p]."""
    out = np.empty((T, C), dtype=np.float32)
    for m in range(N_CORES):
        yf = results[m]["y"].astype(np.float32).reshape(P, G, T_LOC)
        out[m * T_LOC : (m + 1) * T_LOC] = yf.transpose(2, 1, 0).reshape(T_LOC, C)
    return out


def kernel(inputs, state, weight, bias):
    nc = _get_nc()
    in_maps = _prepare_in_maps(inputs, state, weight, bias)
    res = run_bass_kernel_spmd(nc, in_maps, core_ids=list(range(N_CORES)))
    out = _assemble_out(res.results)

    split_points = np.cumsum(OUTPUT_DIMS[:-1]).tolist()
    split_outputs = tuple(
        np.ascontiguousarray(s) for s in np.split(out, split_points, axis=-1)
    )
    updated_state = np.ascontiguousarray(np.asarray(inputs, dtype=np.float32)[T - K : T])
    return (*split_outputs, updated_state)


# revision 32
# speedup vs baseline: 1.1478x; 1.1361x over previous
"""Trainium2 Bass kernel for nn_CausalConv1d (depthwise causal conv, K=4).

Reference computation (T=8192, C=8448, K=4):
    padded = concat([state, inputs], axis=0)            # [T+K, C]
    out[t, c] = bias[c] + sum_j padded[t+j, c] * weight[c, j]
    updated_state = inputs[T-K:T]
    returns (out[:, :8192], out[:, 8192:8320], out[:, 8320:8448], updated_state)

Strategy (measured ~125-130 us/core marginal exec on TRN2, DMA 78 us,
PE-bound; see test.py for methodology):
  - Shard T across the 8 cores: core m computes out rows [m*1024, (m+1)*1024),
    consuming padded rows [m*1024, m*1024+1024+K-1) (halo of K-1=3 rows).
  - fp16 on the wire: inputs are cast to fp16 on the host (10-bit mantissa,
    range-safe for this data) and outputs come back fp16 — halves HBM traffic
    for this memory-regime problem. l2 rel err vs the f32 reference ~3.4e-4.
  - Host pre-permutes each core's input to the exact SBUF layout
    x_flat[p, g*TINP + t] = padded[m*1024 + t, g*128 + p], so every DMA is a
    plain 2D transfer with long contiguous per-partition runs (the naive
    transposed-AP DMA was ~2.4x slower).
  - On device, the depthwise conv runs on the TensorEngine: for each group of
    128 channels, 4 matmuls with lhsT = diag(weight[:, j]) and rhs = the input
    tile read at free-offset j, accumulating the 4 taps into a 2-bank PSUM
    tile. j-outer order so consecutive matmuls share stationary weights
    (fewer LDWEIGHTS). ScalarE then does one 1024-wide PSUM->SBUF copy per
    group with the bias fused (activation Identity, per-partition bias AP).
  - Diagonal weight matrices are built on the fly by the VectorE from a host
    identity matrix via tensor_scalar_mul (per-partition scalar = w[:, j]).
  - 13 of the 66 channel groups (gg % 5 == 4) bypass the PE and are computed
    elementwise (ACT does tap0+bias, DVE does taps 1-3 and the adds) to
    offload the PE, which is otherwise the critical path.
  - Input DMAs issue on the SP HWDGE ring (nc.sync), output DMAs on the ACT
    ring (nc.scalar) so the two streams don't serialize on one FIFO.
  - updated_state is a pure host-side slice of the inputs.
  - A fp32r variant (dtype='f32r': fp32 rounded to 11 mantissa bits, 1 PE
    cycle/row) is kept as a higher-precision fallback (~1.3e-4 l2 rel err,
    ~237 us).
"""

import sys

sys.path.insert(0, "/opt/trn_rl_repo")

from contextlib import ExitStack

import numpy as np

import concourse.bass as bass
import concourse.mybir as mybir
from concourse import bacc
from concourse.tile import TileContext
from concourse.bass_utils import run_bass_kernel_spmd

# Problem shapes (hardcoded per the harness contract).
T, C, K = 8192, 8448, 4
N_CORES = 8
T_LOC = T // N_CORES  # 1024 output rows per core
TIN = T_LOC + K - 1  # 1027 input rows per core
TINP = TIN + 1  # padded per-group stride so tap offsets stay 4B-aligned (fp16)
P = 128  # SBUF partitions
G = C // P  # 66 channel groups
GB = 6  # channel groups per DMA batch
NI = G // GB  # 11 iterations
CHUNK = 512  # PSUM bank = 512 fp32
OUTPUT_DIMS = (8192, 128, 128)

_F32 = mybir.dt.float32
_F32R = mybir.dt.float32r


def _round_fp32r(a: np.ndarray) -> np.ndarray:
    """Round fp32 to 11 mantissa bits (round-to-nearest-even) — exactly what
    the PE's fp32r datapath does to its inputs (verified on hardware)."""
    ai = np.ascontiguousarray(a, dtype=np.float32).view(np.uint32)
    shift = 23 - 11
    bias = ((ai >> np.uint32(shift)) & np.uint32(1)) + np.uint32((1 << (shift - 1)) - 1)
    return (((ai + bias) >> np.uint32(shift)) << np.uint32(shift)).view(np.float32)


def _build(loop_reps: int | None = None, mode: str = "full", dtype: str = "fp16", vec_groups: bool = True, drains: str = "act", gb: int = GB, hints: bool = False, bufs=(3, 3, 16), vec_sel: int = 5, out_eng: str = "act", ps_chunks: int = 2, vec_first: bool = True, vec_tree: str = "chain", diag_eng: str = "dve", vec_m3: str = "dve"):
    """Build the per-core Bass program.

    loop_reps (timing only): wrap the body in a device-side For_i repeat so
    per-exec dispatch overhead can be subtracted out when benchmarking.
    mode: 'full' | 'dma' (skip compute, out-DMA echoes input tile) |
          'compute' (load one tile, run all compute on it, one out-DMA).
    dtype: 'fp16' (x/y/diag in fp16 — half the HBM traffic, 10-bit mantissa)
           or 'f32r' (x/diag fp32r, y fp32 — 11-bit mantissa products).
    """
    nc = bacc.Bacc("TRN2", target_bir_lowering=False, debug=False)

    if dtype == "fp16":
        xdt = ydt = mybir.dt.float16
    else:
        xdt, ydt = _F32R, _F32

    x = nc.dram_tensor("x", [P, G * TINP], xdt, kind="ExternalInput")
    w = nc.dram_tensor("w", [P, G * K], _F32, kind="ExternalInput")
    b = nc.dram_tensor("b", [P, G], _F32, kind="ExternalInput")
    ident = nc.dram_tensor("ident", [P, P], _F32, kind="ExternalInput")
    y = nc.dram_tensor("y", [P, G * T_LOC], ydt, kind="ExternalOutput")

    # Some channel groups are computed elementwise on ACT+DVE instead of the
    # PE, to balance the compute engines (PE alone would be the ceiling).
    # vec groups: gg % 5 == 4 (13 of 66). PE-group drains (PSUM -> SBUF +
    # bias) all go to ACT by default ('split' sends gg % 5 == 0 to DVE).
    def is_vec(gg):
        if not (vec_groups and dtype == "fp16" and mode == "full"):
            return False
        if vec_sel == 10:  # ~20 groups, rotating spread
            return gg % 10 in (1, 4, 7)
        return gg % vec_sel == vec_sel - 1

    def dve_drains(gg):
        return drains == "split" and gg % 5 == 0

    act_t = mybir.ActivationFunctionType

    with TileContext(nc) as tc:
        with (
            tc.tile_pool(name="const", bufs=1) as cpool,
            tc.tile_pool(name="xin", bufs=bufs[0]) as xpool,
            tc.tile_pool(name="yout", bufs=bufs[1]) as ypool,
            tc.tile_pool(name="diag", bufs=bufs[2]) as dpool,
            tc.tile_pool(name="vec", bufs=4) as vpool,
            tc.tile_pool(name="psum", bufs=8 // ps_chunks, space=bass.MemorySpace.PSUM) as pspool,
        ):
            wt = cpool.tile([P, G * K], _F32)
            bt = cpool.tile([P, G], _F32)
            it = cpool.tile([P, P], _F32)
            nc.sync.dma_start(out=wt[:], in_=w[:])
            nc.sync.dma_start(out=bt[:], in_=b[:])
            nc.sync.dma_start(out=it[:], in_=ident[:])

            with ExitStack() as loop_ctx:
                if loop_reps is not None:
                    hint = (
                        (mybir.EngineType.PE, mybir.EngineType.DVE)
                        if hints
                        else ()
                    )
                    loop_ctx.enter_context(
                        tc.For_i(0, loop_reps, 1, hint_engines=hint)
                    )
                for i in range(G // gb):
                    if mode == "compute" and i > 0:
                        xt = xt  # noqa: PLW0127 — reuse first tile
                    else:
                        xt = xpool.tile([P, gb * TINP], xdt)
                        nc.sync.dma_start(
                            out=xt[:], in_=x[:, i * gb * TINP : (i + 1) * gb * TINP]
                        )
                    if mode == "dma":
                        nc.scalar.dma_start(
                            out=y[:, i * gb * T_LOC : (i + 1) * gb * T_LOC],
                            in_=xt[:, : gb * T_LOC].bitcast(ydt),
                        )
                        continue
                    yt = ypool.tile([P, gb * T_LOC], ydt)
                    g_order = sorted(
                        range(gb), key=lambda gx: 0 if is_vec(i * gb + gx) else 1
                    ) if vec_first else range(gb)
                    for g in g_order:
                        gg = i * gb + g
                        base = g * TINP
                        yslc = yt[:, g * T_LOC : (g + 1) * T_LOC]
                        bias_ap = bt[:, gg : gg + 1]
                        if is_vec(gg):
                            # Vector path: 4 taps + bias via ACT+DVE, full
                            # 1024-wide ops to amortize fixed overheads.
                            w_ap = [
                                wt[:, gg * K + j : gg * K + j + 1] for j in range(K)
                            ]
                            xs = [
                                xt[:, base + j : base + j + T_LOC] for j in range(K)
                            ]
                            a0 = vpool.tile([P, T_LOC], ydt, name="a0", tag="a0")
                            nc.scalar.activation(
                                a0[:], xs[0], act_t.Identity,
                                bias=bias_ap, scale=w_ap[0],
                            )
                            a1 = vpool.tile([P, T_LOC], ydt, name="a1", tag="a1")
                            nc.vector.tensor_scalar_mul(a1[:], xs[1], w_ap[1])
                            m2 = vpool.tile([P, T_LOC], ydt, name="m2", tag="m2")
                            nc.vector.tensor_scalar_mul(m2[:], xs[2], w_ap[2])
                            m3 = vpool.tile([P, T_LOC], ydt, name="m3", tag="m3")
                            if vec_m3 == "act":
                                nc.scalar.activation(
                                    m3[:], xs[3], act_t.Copy, scale=w_ap[3]
                                )
                            else:
                                nc.vector.tensor_scalar_mul(m3[:], xs[3], w_ap[3])
                            if vec_tree == "chain":
                                # pure-DVE chain; ACT's a0 joins last so the
                                # cross-engine dependency is off the chain.
                                u = vpool.tile([P, T_LOC], ydt, name="u", tag="s1")
                                nc.vector.tensor_add(u[:], a1[:], m2[:])
                                v = vpool.tile([P, T_LOC], ydt, name="v", tag="s2")
                                nc.vector.tensor_add(v[:], u[:], m3[:])
                                nc.vector.tensor_add(yslc, v[:], a0[:])
                            else:
                                s1 = vpool.tile([P, T_LOC], ydt, name="s1", tag="s1")
                                nc.vector.tensor_add(s1[:], a0[:], a1[:])
                                s2 = vpool.tile([P, T_LOC], ydt, name="s2", tag="s2")
                                nc.vector.tensor_add(s2[:], m2[:], m3[:])
                                nc.vector.tensor_add(yslc, s1[:], s2[:])
                            continue
                        diags = []
                        diag_e = nc.gpsimd if diag_eng == "gpsimd" else nc.vector
                        for j in range(K):
                            d = dpool.tile([P, P], xdt, tag="diag")
                            diag_e.tensor_scalar_mul(
                                d[:], it[:], wt[:, gg * K + j : gg * K + j + 1]
                            )
                            diags.append(d)
                        # PSUM tiles of ps_chunks banks per group; j-outer so
                        # consecutive matmuls share stationary weights (less LDW).
                        pw = ps_chunks * CHUNK
                        pss = [
                            pspool.tile([P, pw], _F32, name="ps", tag="ps")
                            for _ in range(T_LOC // pw)
                        ]
                        for j in range(K):
                            for c in range(T_LOC // CHUNK):
                                ps = pss[c // ps_chunks]
                                pc = c % ps_chunks
                                nc.tensor.matmul(
                                    ps[:, pc * CHUNK : (pc + 1) * CHUNK],
                                    diags[j][:],
                                    xt[:, base + c * CHUNK + j : base + c * CHUNK + j + CHUNK],
                                    start=(j == 0),
                                    stop=(j == K - 1),
                                    skip_group_check=True,
                                )
                        # Wide drains (PSUM+bias -> SBUF), DVE or ACT.
                        for pi, ps in enumerate(pss):
                            ysub = yt[:, g * T_LOC + pi * pw : g * T_LOC + (pi + 1) * pw]
                            if dve_drains(gg):
                                nc.vector.tensor_scalar_add(ysub, ps[:], bias_ap)
                            else:
                                nc.scalar.add(ysub, ps[:], bias_ap)
                    if mode == "compute" and i < G // gb - 1:
                        continue
                    out_dma = nc.gpsimd if out_eng == "gpsimd" else nc.scalar
                    out_dma.dma_start(
                        out=y[:, i * gb * T_LOC : (i + 1) * gb * T_LOC], in_=yt[:]
                    )
    nc.compile()
    return nc


DTYPE = "fp16"  # 'fp16' or 'f32r'

_NC_CACHE = None


def _get_nc():
    global _NC_CACHE
    if _NC_CACHE is None:
        _NC_CACHE = _build(dtype=DTYPE)
    return _NC_CACHE


def _prepare_in_maps(inputs, state, weight, bias):
    inputs = np.asarray(inputs, dtype=np.float32)
    state = np.asarray(state, dtype=np.float32)
    weight = np.asarray(weight, dtype=np.float32)
    bias = np.asarray(bias, dtype=np.float32)

    padded = np.concatenate([state, inputs], axis=0)  # [T+K, C]
    if DTYPE == "fp16":
        padded_r = padded.astype(np.float16)
        w_r = weight  # diag build rounds to fp16 on the DVE write
    else:
        padded_r = _round_fp32r(padded)
        w_r = _round_fp32r(weight)

    w_sb = np.ascontiguousarray(
        w_r.reshape(G, P, K).transpose(1, 0, 2).reshape(P, G * K)
    )
    b_sb = np.ascontiguousarray(bias.reshape(G, P).T)  # [P, G]
    ident = np.eye(P, dtype=np.float32)

    in_maps = []
    for m in range(N_CORES):
        seg = padded_r[m * T_LOC : m * T_LOC + TIN]  # [TIN, C]
        # x_flat[p, g*TINP + t] = seg[t, g*128 + p]  (last TINP-TIN cols zero pad)
        xm = np.zeros((P, G, TINP), dtype=padded_r.dtype)
        xm[:, :, :TIN] = seg.reshape(TIN, G, P).transpose(2, 1, 0)
        in_maps.append(
            {"x": xm.reshape(P, G * TINP), "w": w_sb, "b": b_sb, "ident": ident}
        )
    return in_maps


def _assemble_out(results):
    """results[m]['y'] is [P, G*T_LOC] with y[p, g*T_LOC + t] = out[m*T_LOC+t, g*128+p]."""
    out = np.empty((T, C), dtype=np.float32)
    for m in range(N_CORES):
        yf = results[m]["y"].astype(np.float32).reshape(P, G, T_LOC)
        out[m * T_LOC : (m + 1) * T_LOC] = yf.transpose(2, 1, 0).reshape(T_LOC, C)
    return out


def kernel(inputs, state, weight, bias):
    nc = _get_nc()
    in_maps = _prepare_in_maps(inputs, state, weight, bias)
    res = run_bass_kernel_spmd(nc, in_maps, core_ids=list(range(N_CORES)))
    out = _assemble_out(res.results)

    split_points = np.cumsum(OUTPUT_DIMS[:-1]).tolist()
    split_outputs = tuple(
        np.ascontiguousarray(s) for s in np.split(out, split_points, axis=-1)
    )
    updated_state = np.ascontiguousarray(np.asarray(inputs, dtype=np.float32)[T - K : T])
    return (*split_outputs, updated_state)
